# revision 1
# baseline (speedup 1.0000x reference)
"""Causal single-head attention 1D (B=4, C=512, T=4096) on 8 TRN2 NeuronCores.

Sharding: data-parallel over (batch, query-half). Each of the 8 cores handles
one batch b = core//2 and one query-half h = core%2, where the query half is
the h-th 256-wide slice of every 512-wide query chunk (8 chunks over T=4096).
This makes the causal block structure identical on every core (SPMD-friendly)
while balancing the triangular causal work exactly.

Per core:
  phase 1: stream x[b] and build K ([c, s] layout) and V ([s, c] layout) in
           SBUF, resident for the whole kernel (16 MB of the ~26 MB SBUF).
  phase 2: for each of 8 query chunks (256 queries): project Q, then loop over
           the causally-needed 128-wide key tiles: S^T = K_tile^T-layout matmul
           against Q, exp (no max subtraction -- scores are O(1) by
           construction), causal mask multiply on the 4 diagonal tiles,
           accumulate V^T @ E and the ones-row sums in PSUM, then normalize,
           project with Wp and add the residual.

All matmuls run in float32r (full-rate fp32 mode of the PE).
"""

import numpy as np

import concourse.bass as bass
import concourse.bacc as bacc
import concourse.mybir as mybir
from concourse import tile
from concourse.bass_utils import run_bass_kernel_spmd
from contextlib import ExitStack

B, C, T = 4, 512, 4096
NCORE = 8
P = 128
CT = C // P            # 4 channel tiles
NCH = T // 512         # 8 query chunks of 512
SUB = 256              # per-core queries per chunk
TQ = NCH * SUB         # 2048 queries per core
NST = T // P           # 32 key tiles
SCALE = float(C) ** -0.5

f32 = mybir.dt.float32
f32r = mybir.dt.float32r
AF = mybir.ActivationFunctionType
ts = bass.ts


def _build_program():
    nc = bacc.Bacc("TRN2", target_bir_lowering=False, debug=False,
                   num_devices=NCORE)

    xb = nc.dram_tensor("xb", [CT, P, T], f32, kind="ExternalInput")
    xq = nc.dram_tensor("xq", [CT, P, TQ], f32, kind="ExternalInput")
    xqb = nc.dram_tensor("xqb", [CT, P, TQ], f32, kind="ExternalInput")
    wqt = nc.dram_tensor("wqt", [CT, P, C], f32, kind="ExternalInput")
    wkt = nc.dram_tensor("wkt", [CT, P, C], f32, kind="ExternalInput")
    wvt = nc.dram_tensor("wvt", [CT, P, C], f32, kind="ExternalInput")
    wpt = nc.dram_tensor("wpt", [CT, P, C], f32, kind="ExternalInput")
    bqd = nc.dram_tensor("bqd", [CT, P, 1], f32, kind="ExternalInput")
    bkd = nc.dram_tensor("bkd", [CT, P, 1], f32, kind="ExternalInput")
    bvd = nc.dram_tensor("bvd", [P, C], f32, kind="ExternalInput")
    mkd = nc.dram_tensor("mkd", [4, P, SUB], f32, kind="ExternalInput")
    onc = nc.dram_tensor("onc", [P, 1], f32, kind="ExternalInput")
    onr = nc.dram_tensor("onr", [1, P], f32, kind="ExternalInput")
    out = nc.dram_tensor("out", [CT, P, TQ], f32, kind="ExternalOutput")

    with tile.TileContext(nc) as tc, ExitStack() as ctx:
        const = ctx.enter_context(tc.tile_pool(name="const", bufs=1))

        wq_sb = const.tile([P, CT, C], f32r, tag="wq")
        wp_sb = const.tile([P, CT, C], f32r, tag="wp")
        k_sb = const.tile([P, CT, T], f32r, tag="k")
        v_sb = const.tile([P, NST, C], f32r, tag="v")
        mask_sb = const.tile([P, 4, SUB], f32r, tag="mask")
        bvb_sb = const.tile([P, C], f32, tag="bvb")
        bq_sb = const.tile([P, CT], f32, tag="bq")
        bk_sb = const.tile([P, CT], f32, tag="bk")
        ones_c = const.tile([P, 1], f32r, tag="onec")
        ones_r = const.tile([1, P], f32r, tag="oner")

        # phase-2 constants, loaded while phase-1 compute runs (dribbled a
        # few per s-chunk so they never sit ahead of the critical x loads)
        late_dmas = []
        for j in range(CT):
            late_dmas.append(lambda j=j: nc.sync.dma_start(
                wq_sb[:, j, :], wqt[j].bitcast(f32r)))
        for j in range(CT):
            late_dmas.append(lambda j=j: nc.sync.dma_start(
                wp_sb[:, j, :], wpt[j].bitcast(f32r)))
        for m in range(4):
            late_dmas.append(lambda m=m: nc.sync.dma_start(
                mask_sb[:, m, :], mkd[m].bitcast(f32r)))
        for j in range(CT):
            late_dmas.append(lambda j=j: nc.sync.dma_start(
                bq_sb[:, j:j + 1], bqd[j]))
        late_dmas.append(lambda: nc.sync.dma_start(
            ones_c[:], onc[:].bitcast(f32r)))
        late_dmas.append(lambda: nc.sync.dma_start(
            ones_r[:], onr[:].bitcast(f32r)))

        pp = ctx.enter_context(tc.tile_pool(name="pp", bufs=3, space="PSUM"))
        ph = ctx.enter_context(tc.tile_pool(name="ph", bufs=1, space="PSUM"))

        # ---- phase 1: K and V resident in SBUF --------------------------
        with tc.tile_pool(name="xp", bufs=2) as xp, \
             tc.tile_pool(name="wkv", bufs=1) as wkv:
            wk_sb = wkv.tile([P, CT, C], f32r, tag="wk")
            wv_sb = wkv.tile([P, CT, C], f32r, tag="wv")
            for j in range(CT):
                nc.sync.dma_start(wk_sb[:, j, :], wkt[j].bitcast(f32r))
            for sc in range(NCH):
                xt = xp.tile([P, CT, 512], f32r, tag="xt")
                for j in range(CT):
                    nc.sync.dma_start(xt[:, j, :],
                                      xb[j][:, ts(sc, 512)].bitcast(f32r))
                if sc == 0:
                    # needed from the first V tile onward; after the first
                    # x chunk so the very first K matmul starts asap
                    for j in range(CT):
                        nc.sync.dma_start(wv_sb[:, j, :],
                                          wvt[j].bitcast(f32r))
                    for j in range(CT):
                        nc.sync.dma_start(bk_sb[:, j:j + 1], bkd[j])
                    nc.sync.dma_start(bvb_sb[:], bvd[:])
                for o in range(CT):
                    pk = pp.tile([P, 512], f32, tag="mm")
                    for cj in range(CT):
                        nc.tensor.matmul(pk[:], wk_sb[:, cj, ts(o, P)],
                                         xt[:, cj, :],
                                         start=(cj == 0), stop=(cj == CT - 1))
                    nc.scalar.activation(k_sb[:, o, ts(sc, 512)], pk[:],
                                         AF.Identity, bias=bk_sb[:, o:o + 1])
                for ss in range(4):
                    pv = pp.tile([P, 512], f32, tag="mm")
                    for cj in range(CT):
                        nc.tensor.matmul(pv[:], xt[:, cj, ts(ss, P)],
                                         wv_sb[:, cj, :],
                                         start=(cj == 0), stop=(cj == CT - 1))
                    nc.vector.tensor_add(v_sb[:, sc * 4 + ss, :], pv[:],
                                         bvb_sb[:])
                if sc >= 1:
                    for _ in range(4):
                        if late_dmas:
                            late_dmas.pop(0)()
            while late_dmas:
                late_dmas.pop(0)()

        # ---- phase 2: attention per query chunk, pipelined across chunks
        with tc.tile_pool(name="qp", bufs=2) as qp, \
             tc.tile_pool(name="ep", bufs=4) as ep, \
             tc.tile_pool(name="op", bufs=2) as op:

            chunk_tiles = {}

            def load_and_qproj(c):
                qx = qp.tile([P, CT, SUB], f32r, tag="qx", name="qx")
                xr = qp.tile([P, CT, SUB], f32, tag="xr", name="xr")
                for j in range(CT):
                    nc.sync.dma_start(qx[:, j, :],
                                      xq[j][:, ts(c, SUB)].bitcast(f32r))
                    nc.sync.dma_start(xr[:, j, :], xqb[j][:, ts(c, SUB)])
                q_sb = qp.tile([P, CT, SUB], f32r, tag="q", name="q_sb")
                for o in range(CT):
                    pq = pp.tile([P, SUB], f32, tag="mm", name="pq")
                    for cj in range(CT):
                        nc.tensor.matmul(pq[:], wq_sb[:, cj, ts(o, P)],
                                         qx[:, cj, :],
                                         start=(cj == 0), stop=(cj == CT - 1))
                    nc.scalar.activation(q_sb[:, o, :], pq[:], AF.Identity,
                                         bias=bq_sb[:, o:o + 1])
                chunk_tiles[c] = (q_sb, xr)

            def s_loop(c):
                q_sb, _ = chunk_tiles[c]
                ntr = 4 * c + 4
                ht = [ph.tile([P, SUB], f32, tag=f"ht{cs}", name=f"ht{cs}")
                      for cs in range(CT)]
                sm = ph.tile([1, SUB], f32, tag="sm", name="sm")
                st_tiles = {}

                def qk(k):
                    stp = pp.tile([P, SUB], f32, tag="mm", name="stp")
                    for cj in range(CT):
                        nc.tensor.matmul(stp[:], k_sb[:, cj, ts(k, P)],
                                         q_sb[:, cj, :],
                                         start=(cj == 0), stop=(cj == CT - 1))
                    st_tiles[k] = stp

                qk(0)
                for k in range(ntr):
                    if k + 1 < ntr:
                        qk(k + 1)
                    stp = st_tiles.pop(k)
                    et = ep.tile([P, SUB], f32r, tag="et", name="et")
                    nc.scalar.activation(et[:], stp[:], AF.Exp, scale=SCALE)
                    if k >= 4 * c:
                        nc.vector.tensor_mul(et[:], et[:],
                                             mask_sb[:, k - 4 * c, :])
                    for cs in range(CT):
                        nc.tensor.matmul(ht[cs][:], v_sb[:, k, ts(cs, P)],
                                         et[:], start=(k == 0),
                                         stop=(k == ntr - 1))
                    nc.tensor.matmul(sm[:], ones_c[:], et[:],
                                     start=(k == 0), stop=(k == ntr - 1))
                return ht, sm

            def finish(c, ht, sm):
                _, xr = chunk_tiles.pop(c)
                r_sb = op.tile([1, SUB], f32r, tag="rsb", name="r_sb")
                with nc.allow_low_precision(reason="float32r is fp32-width"):
                    nc.vector.reciprocal(r_sb[:], sm[:])
                prb = pp.tile([P, SUB], f32, tag="mm", name="prb")
                nc.tensor.matmul(prb[:], ones_r[:], r_sb[:], start=True,
                                 stop=True)
                r_b = op.tile([P, SUB], f32, tag="rb", name="r_b")
                nc.scalar.activation(r_b[:], prb[:], AF.Identity)

                hs = qp.tile([P, CT, SUB], f32r, tag="hs", name="hs")
                for cs in range(CT):
                    nc.scalar.activation(hs[:, cs, :], ht[cs][:], AF.Identity)

                for o in range(CT):
                    pu = pp.tile([P, SUB], f32, tag="mm", name="pu")
                    for cj in range(CT):
                        nc.tensor.matmul(pu[:], wp_sb[:, cj, ts(o, P)],
                                         hs[:, cj, :],
                                         start=(cj == 0), stop=(cj == CT - 1))
                    og = op.tile([P, SUB], f32, tag="og", name="og")
                    nc.vector.tensor_mul(og[:], pu[:], r_b[:])
                    nc.vector.tensor_add(og[:], og[:], xr[:, o, :])
                    nc.sync.dma_start(out[o][:, ts(c, SUB)], og[:])

            load_and_qproj(0)
            for c in range(NCH):
                ht, sm = s_loop(c)
                if c + 1 < NCH:
                    # next chunk's Q projection keeps the PE busy while this
                    # chunk's reciprocal/copies run on DVE/ACT
                    load_and_qproj(c + 1)
                finish(c, ht, sm)

    nc.finalize()
    return nc


def _masks(h):
    m = np.zeros((4, P, SUB), np.float32)
    f = np.arange(SUB)[None, :]
    p = np.arange(P)[:, None]
    for k in range(4):
        d = 128 * k - 256 * h
        m[k] = (f >= p + d).astype(np.float32)
    return m


def _in_maps(inputs):
    x = np.asarray(inputs["x"], np.float32)
    Wq = np.asarray(inputs["Wq"], np.float32)
    bq = np.asarray(inputs["bq"], np.float32)
    Wk = np.asarray(inputs["Wk"], np.float32)
    bk = np.asarray(inputs["bk"], np.float32)
    Wv = np.asarray(inputs["Wv"], np.float32)
    bv = np.asarray(inputs["bv"], np.float32)
    Wp = np.asarray(inputs["Wp"], np.float32)
    bp = np.asarray(inputs["bp"], np.float32)

    common = {
        "wqt": np.ascontiguousarray(Wq.T.reshape(CT, P, C)),
        "wkt": np.ascontiguousarray(Wk.T.reshape(CT, P, C)),
        "wvt": np.ascontiguousarray(Wv.T.reshape(CT, P, C)),
        "wpt": np.ascontiguousarray(Wp.T.reshape(CT, P, C)),
        "bqd": np.ascontiguousarray(bq.reshape(CT, P, 1)),
        "bkd": np.ascontiguousarray(bk.reshape(CT, P, 1)),
        "bvd": np.ascontiguousarray(np.broadcast_to(bv[None, :], (P, C))),
        "onc": np.ones((P, 1), np.float32),
        "onr": np.ones((1, P), np.float32),
    }
    maps = []
    for core in range(NCORE):
        b, h = divmod(core, 2)
        cols = (np.arange(NCH)[:, None] * 512 + h * SUB
                + np.arange(SUB)[None, :]).ravel()
        xg = x[b][:, cols]                      # [C, TQ] gathered queries
        m = dict(common)
        m["xb"] = np.ascontiguousarray(x[b].reshape(CT, P, T))
        m["xq"] = np.ascontiguousarray(xg.reshape(CT, P, TQ))
        m["xqb"] = np.ascontiguousarray((xg + bp[:, None]).reshape(CT, P, TQ))
        m["mkd"] = _masks(h)
        maps.append((m, b, cols))
    return maps


_prog_cache = {}


def _get_program():
    if "nc" not in _prog_cache:
        _prog_cache["nc"] = _build_program()
    return _prog_cache["nc"]


def kernel(**inputs):
    return _run(inputs, trace=False)[0]


def _run(inputs, trace=False):
    nc = _get_program()
    maps = _in_maps(inputs)
    res = run_bass_kernel_spmd(nc, [m for m, _, _ in maps],
                               core_ids=list(range(NCORE)), trace=trace)
    x = np.asarray(inputs["x"], np.float32)
    full = np.empty((B, C, T), np.float32)
    for core, (_, b, cols) in enumerate(maps):
        full[b][:, cols] = res.results[core]["out"].reshape(C, TQ)
    return full, res



# revision 13
# speedup vs baseline: 1.1552x; 1.1552x over previous
"""Causal single-head attention 1D (B=4, C=512, T=4096) on 8 TRN2 NeuronCores.

Sharding: data-parallel over (batch, query-half). Each of the 8 cores handles
one batch b = core//2 and one query-half h = core%2, where the query half is
the h-th 256-wide slice of every 512-wide query chunk (8 chunks over T=4096).
This makes the causal block structure identical on every core (SPMD-friendly)
while balancing the triangular causal work.

Per core:
  phase 1: stream x[b] and build K ([c, s] layout, f32r) and V ([s, c]
           layout, bf16) in SBUF, resident for the whole kernel.
  phase 2: for each of 8 query chunks (256 queries): project Q, then loop
           over the causally-needed 128-wide key tiles: S^T = K_tile matmul
           against Q, exp on ScalarE (bf16 out, no max subtraction --
           scores are O(1) by construction), causal mask multiply on the 4
           diagonal tiles (DVE), softmax denominator accumulated on the DVE
           (keeps the PE free), V^T @ E accumulated in PSUM. Normalization
           is a DVE reciprocal of the summed denominator, broadcast across
           partitions with a K=1 matmul, then the Wp projection, residual
           and DMA out.

The output bias bp is folded on the host: the streamed query tile is x + bp
(used directly for the residual) and the Q bias is adjusted to
bq' = bq - Wq @ bp so the Q projection still effectively sees plain x.
"""

import ml_dtypes
import numpy as np

import concourse.bass as bass
import concourse.bacc as bacc
import concourse.mybir as mybir
from concourse import tile
from concourse.bass_utils import run_bass_kernel_spmd
from contextlib import ExitStack

B, C, T = 4, 512, 4096
NCORE = 8
P = 128
CT = C // P            # 4 channel tiles
NCH = T // 512         # 8 query chunks of 512
SUB = 256              # per-core queries per chunk
TQ = NCH * SUB         # 2048 queries per core
NST = T // P           # 32 key tiles
SCALE = float(C) ** -0.5

f32 = mybir.dt.float32
f32r = mybir.dt.float32r
bf16 = mybir.dt.bfloat16
AF = mybir.ActivationFunctionType
ts = bass.ts


def _build_program():
    nc = bacc.Bacc("TRN2", target_bir_lowering=False, debug=False,
                   num_devices=NCORE)

    xb = nc.dram_tensor("xb", [CT, P, T], f32, kind="ExternalInput")
    xq = nc.dram_tensor("xq", [CT, P, TQ], f32, kind="ExternalInput")
    wqt = nc.dram_tensor("wqt", [CT, P, C], f32, kind="ExternalInput")
    wkt = nc.dram_tensor("wkt", [CT, P, C], f32, kind="ExternalInput")
    wvt = nc.dram_tensor("wvt", [CT, P, C], f32, kind="ExternalInput")
    wpt = nc.dram_tensor("wpt", [CT, P, C], f32, kind="ExternalInput")
    bqd = nc.dram_tensor("bqd", [CT, P, 1], f32, kind="ExternalInput")
    bkd = nc.dram_tensor("bkd", [CT, P, 1], f32, kind="ExternalInput")
    bvd = nc.dram_tensor("bvd", [P, C], f32, kind="ExternalInput")
    mkd = nc.dram_tensor("mkd", [4, P, SUB], bf16, kind="ExternalInput")
    onc = nc.dram_tensor("onc", [P, 1], f32, kind="ExternalInput")
    nonr = nc.dram_tensor("nonr", [1, P], f32, kind="ExternalInput")
    out = nc.dram_tensor("out", [CT, P, TQ], f32, kind="ExternalOutput")

    with tile.TileContext(nc) as tc, ExitStack() as ctx:
        const = ctx.enter_context(tc.tile_pool(name="const", bufs=1))

        wq_sb = const.tile([P, CT, C], f32r, tag="wq")
        wp_sb = const.tile([P, CT, C], f32r, tag="wp")
        k_sb = const.tile([P, CT, T], f32r, tag="k")
        v_sb = const.tile([P, NST, C], bf16, tag="v")
        mask_sb = const.tile([P, 4, SUB], bf16, tag="mask")
        bvb_sb = const.tile([P, C], f32, tag="bvb")
        bq_sb = const.tile([P, CT], f32, tag="bq")
        bk_sb = const.tile([P, CT], f32, tag="bk")
        ones_c = const.tile([P, 1], f32r, tag="onec")
        onr_sb = const.tile([1, P], f32r, tag="onr")

        # PSUM allocates whole 2KB banks per buffer: mm 4 + ht 4 = 8 banks.
        # One deep rotating pool for every non-ht matmul output avoids
        # buffer-reuse waits between the finish tail and the next chunk.
        pp = ctx.enter_context(tc.tile_pool(name="pp", bufs=4, space="PSUM"))
        ph = ctx.enter_context(tc.tile_pool(name="ph", bufs=1, space="PSUM"))

        qp = ctx.enter_context(tc.tile_pool(name="qp", bufs=2))
        ep = ctx.enter_context(tc.tile_pool(name="ep", bufs=10))
        hp = ctx.enter_context(tc.tile_pool(name="hp", bufs=1))
        op = ctx.enter_context(tc.tile_pool(name="op", bufs=2))
        ogp = ctx.enter_context(tc.tile_pool(name="ogp", bufs=3))

        # phase-2 constants, loaded while phase-1 compute runs (dribbled a
        # few per s-chunk so they never sit ahead of the critical x loads)
        late_dmas = []
        for j in range(CT):
            late_dmas.append(lambda j=j: nc.sync.dma_start(
                wq_sb[:, j, :], wqt[j].bitcast(f32r)))
        for j in range(CT):
            late_dmas.append(lambda j=j: nc.sync.dma_start(
                bq_sb[:, j:j + 1], bqd[j]))
        for m in range(4):
            late_dmas.append(lambda m=m: nc.sync.dma_start(
                mask_sb[:, m, :], mkd[m]))
        late_dmas.append(lambda: nc.sync.dma_start(
            ones_c[:], onc[:].bitcast(f32r)))
        late_dmas.append(lambda: nc.sync.dma_start(
            onr_sb[:], nonr[:].bitcast(f32r)))
        for j in range(CT):
            late_dmas.append(lambda j=j: nc.sync.dma_start(
                wp_sb[:, j, :], wpt[j].bitcast(f32r)))

        chunk_tiles = {}

        def load_q(c):
            qx = qp.tile([P, CT, SUB], f32r, tag="qx", name="qx")
            for j in range(CT):
                nc.sync.dma_start(qx[:, j, :],
                                  xq[j][:, ts(c, SUB)].bitcast(f32r))
            q_sb = qp.tile([P, CT, SUB], f32r, tag="q", name="q_sb")
            for o in range(CT):
                pq = pp.tile([P, SUB], f32, tag="mm", name="pq")
                for cj in range(CT):
                    nc.tensor.matmul(pq[:], wq_sb[:, cj, ts(o, P)],
                                     qx[:, cj, :],
                                     start=(cj == 0), stop=(cj == CT - 1))
                nc.scalar.activation(q_sb[:, o, :], pq[:], AF.Identity,
                                     bias=bq_sb[:, o:o + 1])
            chunk_tiles[c] = (q_sb, qx)

        # ---- phase 1: K and V resident in SBUF --------------------------
        with tc.tile_pool(name="xp", bufs=2) as xp, \
             tc.tile_pool(name="wkv", bufs=1) as wkv:
            wk_sb = wkv.tile([P, CT, C], f32r, tag="wk")
            wv_sb = wkv.tile([P, CT, C], f32r, tag="wv")
            for j in range(CT):
                nc.sync.dma_start(wk_sb[:, j, :], wkt[j].bitcast(f32r))
            for sc in range(NCH):
                xt = xp.tile([P, CT, 512], f32r, tag="xt")
                for j in range(CT):
                    nc.sync.dma_start(xt[:, j, :],
                                      xb[j][:, ts(sc, 512)].bitcast(f32r))
                if sc == 0:
                    # needed from the first V tile onward; after the first
                    # x chunk so the very first K matmul starts asap
                    for j in range(CT):
                        nc.sync.dma_start(wv_sb[:, j, :],
                                          wvt[j].bitcast(f32r))
                    for j in range(CT):
                        nc.sync.dma_start(bk_sb[:, j:j + 1], bkd[j])
                    nc.sync.dma_start(bvb_sb[:], bvd[:])
                for o in range(CT):
                    pk = pp.tile([P, 512], f32, tag="mm", name="pk")
                    for cj in range(CT):
                        nc.tensor.matmul(pk[:], wk_sb[:, cj, ts(o, P)],
                                         xt[:, cj, :],
                                         start=(cj == 0), stop=(cj == CT - 1))
                    nc.scalar.activation(k_sb[:, o, ts(sc, 512)], pk[:],
                                         AF.Identity, bias=bk_sb[:, o:o + 1])
                for ss in range(4):
                    pv = pp.tile([P, 512], f32, tag="mm", name="pv")
                    for cj in range(CT):
                        nc.tensor.matmul(pv[:], xt[:, cj, ts(ss, P)],
                                         wv_sb[:, cj, :],
                                         start=(cj == 0), stop=(cj == CT - 1))
                    nc.vector.tensor_add(v_sb[:, sc * 4 + ss, :], pv[:],
                                         bvb_sb[:])
                if sc >= 1:
                    for _ in range(4):
                        if late_dmas:
                            late_dmas.pop(0)()
                if sc == NCH - 2:
                    while late_dmas:
                        late_dmas.pop(0)()
                    # chunk-0 query prefetch + projection: the DMA lands and
                    # the matmuls queue behind the remaining phase-1 work,
                    # so phase 2 starts with zero PE idle
                    load_q(0)

        # ---- phase 2: attention per query chunk, pipelined across chunks
        def s_loop(c):
            q_sb, _ = chunk_tiles[c]
            ntr = 4 * c + 4
            ht = [ph.tile([P, SUB], f32, tag=f"ht{cs}", name=f"ht{cs}")
                  for cs in range(CT)]
            acc = op.tile([P, SUB], f32r, tag="acc", name="acc")
            st_tiles = {}

            def qk(k):
                stp = pp.tile([P, SUB], f32, tag="mm", name="stp")
                for cj in range(CT):
                    nc.tensor.matmul(stp[:], k_sb[:, cj, ts(k, P)],
                                     q_sb[:, cj, :],
                                     start=(cj == 0), stop=(cj == CT - 1))
                st_tiles[k] = stp

            qk(0)
            for k in range(ntr):
                if k + 1 < ntr:
                    qk(k + 1)
                stp = st_tiles.pop(k)
                et = ep.tile([P, SUB], bf16, tag="et", name="et")
                nc.scalar.activation(et[:], stp[:], AF.Exp, scale=SCALE)
                if k >= 4 * c:
                    nc.vector.tensor_mul(et[:], et[:],
                                         mask_sb[:, k - 4 * c, :])
                if k == 0:
                    nc.vector.tensor_copy(acc[:], et[:])
                else:
                    nc.vector.tensor_add(acc[:], acc[:], et[:])
                for cs in range(CT):
                    nc.tensor.matmul(ht[cs][:], v_sb[:, k, ts(cs, P)],
                                     et[:], start=(k == 0),
                                     stop=(k == ntr - 1))
            # drain ht -> SBUF right away (queued on ACT before the next
            # chunk's q-bias ops) so the out projection never waits
            hs = hp.tile([P, CT, SUB], f32r, tag="hs", name="hs")
            for cs in range(CT):
                nc.scalar.activation(hs[:, cs, :], ht[cs][:], AF.Identity)
            return hs, acc

        def finish_head(c, acc):
            # softmax denominator: partition-sum of acc via a ones matmul,
            # then reciprocal on DVE (emitted before load_q so it overlaps
            # the next chunk's Q projection on the PE)
            sm = pp.tile([1, SUB], f32, tag="mm", name="sm")
            nc.tensor.matmul(sm[:], ones_c[:], acc[:],
                             start=True, stop=True)
            r_sb = op.tile([1, SUB], f32r, tag="r", name="r_sb")
            with nc.allow_low_precision(reason="float32r is fp32-width"):
                nc.vector.reciprocal(r_sb[:], sm[:])
            return r_sb

        def finish(c, hs, r_sb):
            _, qx = chunk_tiles.pop(c)
            prb = pp.tile([P, SUB], f32, tag="mm", name="prb")
            nc.tensor.matmul(prb[:], onr_sb[:], r_sb[:], start=True,
                             stop=True)
            r_b = op.tile([P, SUB], f32, tag="rb", name="r_b")
            nc.scalar.activation(r_b[:], prb[:], AF.Identity)

            for o in range(CT):
                pu = pp.tile([P, SUB], f32, tag="mm", name="pu")
                for cj in range(CT):
                    nc.tensor.matmul(pu[:], wp_sb[:, cj, ts(o, P)],
                                     hs[:, cj, :],
                                     start=(cj == 0), stop=(cj == CT - 1))
                og = ogp.tile([P, SUB], f32, tag="og", name="og")
                nc.vector.tensor_mul(og[:], pu[:], r_b[:])
                nc.vector.tensor_add(og[:], og[:], qx[:, o, :].bitcast(f32))
                nc.sync.dma_start(out[o][:, ts(c, SUB)], og[:])

        for c in range(NCH):
            hs, acc = s_loop(c)
            r_sb = finish_head(c, acc)
            if c + 1 < NCH:
                # next chunk's Q projection keeps the PE busy while the
                # denominator drains through DVE
                load_q(c + 1)
            finish(c, hs, r_sb)

    nc.finalize()
    return nc


def _masks(h):
    m = np.zeros((4, P, SUB), np.float32)
    f = np.arange(SUB)[None, :]
    p = np.arange(P)[:, None]
    for k in range(4):
        d = 128 * k - 256 * h
        m[k] = (f >= p + d).astype(np.float32)
    return m.astype(ml_dtypes.bfloat16)


def _in_maps(inputs):
    x = np.asarray(inputs["x"], np.float32)
    Wq = np.asarray(inputs["Wq"], np.float64)
    bq = np.asarray(inputs["bq"], np.float64)
    Wk = np.asarray(inputs["Wk"], np.float32)
    bk = np.asarray(inputs["bk"], np.float32)
    Wv = np.asarray(inputs["Wv"], np.float32)
    bv = np.asarray(inputs["bv"], np.float32)
    Wp = np.asarray(inputs["Wp"], np.float32)
    bp = np.asarray(inputs["bp"], np.float64)

    bqp = (bq - Wq @ bp).astype(np.float32)   # bq' = bq - Wq bp
    Wq32 = Wq.astype(np.float32)
    bp32 = bp.astype(np.float32)

    common = {
        "wqt": np.ascontiguousarray(Wq32.T.reshape(CT, P, C)),
        "wkt": np.ascontiguousarray(Wk.T.reshape(CT, P, C)),
        "wvt": np.ascontiguousarray(Wv.T.reshape(CT, P, C)),
        "wpt": np.ascontiguousarray(Wp.T.reshape(CT, P, C)),
        "bqd": np.ascontiguousarray(bqp.reshape(CT, P, 1)),
        "bkd": np.ascontiguousarray(bk.reshape(CT, P, 1)),
        "bvd": np.ascontiguousarray(np.broadcast_to(bv[None, :], (P, C))),
        "onc": np.ones((P, 1), np.float32),
        "nonr": np.ones((1, P), np.float32),
    }
    maps = []
    for core in range(NCORE):
        b, h = divmod(core, 2)
        cols = (np.arange(NCH)[:, None] * 512 + h * SUB
                + np.arange(SUB)[None, :]).ravel()
        xg = x[b][:, cols] + bp32[:, None]      # [C, TQ] queries + bp
        m = dict(common)
        m["xb"] = np.ascontiguousarray(x[b].reshape(CT, P, T))
        m["xq"] = np.ascontiguousarray(xg.reshape(CT, P, TQ))
        m["mkd"] = _masks(h)
        maps.append((m, b, cols))
    return maps


_prog_cache = {}


def _get_program():
    if "nc" not in _prog_cache:
        _prog_cache["nc"] = _build_program()
    return _prog_cache["nc"]


def kernel(**inputs):
    return _run(inputs, trace=False)[0]


def _run(inputs, trace=False):
    nc = _get_program()
    maps = _in_maps(inputs)
    res = run_bass_kernel_spmd(nc, [m for m, _, _ in maps],
                               core_ids=list(range(NCORE)), trace=trace)
    x = np.asarray(inputs["x"], np.float32)
    full = np.empty((B, C, T), np.float32)
    for core, (_, b, cols) in enumerate(maps):
        full[b][:, cols] = res.results[core]["out"].reshape(C, TQ)
    return full, res


# revision 14
# speedup vs baseline: 1.1815x; 1.0228x over previous
"""Causal single-head attention 1D (B=4, C=512, T=4096) on 8 TRN2 NeuronCores.

Sharding: data-parallel over (batch, query-half). Each of the 8 cores handles
one batch b = core//2 and one query-half h = core%2, where the query half is
the h-th 256-wide slice of every 512-wide query chunk (8 chunks over T=4096).
This makes the causal block structure identical on every core (SPMD-friendly)
while balancing the triangular causal work.

Per core:
  phase 1: stream x[b] and build K ([c, s] layout, f32r) and V ([s, c]
           layout, bf16) in SBUF, resident for the whole kernel.
  phase 2: for each of 8 query chunks (256 queries): project Q, then loop
           over the causally-needed 128-wide key tiles: S^T = K_tile matmul
           against Q, exp on ScalarE (bf16 out, no max subtraction --
           scores are O(1) by construction), causal mask multiply on the 4
           diagonal tiles (DVE), softmax denominator accumulated on the DVE
           (keeps the PE free), V^T @ E accumulated in PSUM. Normalization
           is a DVE reciprocal of the summed denominator, broadcast across
           partitions with a K=1 matmul, then the Wp projection, residual
           and DMA out.

The output bias bp is folded on the host: the streamed query tile is x + bp
(used directly for the residual) and the Q bias is adjusted to
bq' = bq - Wq @ bp so the Q projection still effectively sees plain x.
"""

import ml_dtypes
import numpy as np

import concourse.bass as bass
import concourse.bacc as bacc
import concourse.mybir as mybir
from concourse import tile
from concourse.bass_utils import run_bass_kernel_spmd
from contextlib import ExitStack

B, C, T = 4, 512, 4096
NCORE = 8
P = 128
CT = C // P            # 4 channel tiles
NCH = T // 512         # 8 query chunks of 512
SUB = 256              # per-core queries per chunk
TQ = NCH * SUB         # 2048 queries per core
NST = T // P           # 32 key tiles
SCALE = float(C) ** -0.5

f32 = mybir.dt.float32
f32r = mybir.dt.float32r
bf16 = mybir.dt.bfloat16
AF = mybir.ActivationFunctionType
ts = bass.ts


def _build_program():
    nc = bacc.Bacc("TRN2", target_bir_lowering=False, debug=False,
                   num_devices=NCORE)

    xb = nc.dram_tensor("xb", [P, CT, T], f32, kind="ExternalInput")
    xq = nc.dram_tensor("xq", [P, CT, TQ], f32, kind="ExternalInput")
    wqt = nc.dram_tensor("wqt", [P, CT, C], f32, kind="ExternalInput")
    wkt = nc.dram_tensor("wkt", [P, CT, C], f32, kind="ExternalInput")
    wvt = nc.dram_tensor("wvt", [P, CT, C], f32, kind="ExternalInput")
    wpt = nc.dram_tensor("wpt", [P, CT, C], f32, kind="ExternalInput")
    bqd = nc.dram_tensor("bqd", [P, CT], f32, kind="ExternalInput")
    bkd = nc.dram_tensor("bkd", [P, CT], f32, kind="ExternalInput")
    bvd = nc.dram_tensor("bvd", [P, C], f32, kind="ExternalInput")
    mkd = nc.dram_tensor("mkd", [4, P, SUB], bf16, kind="ExternalInput")
    onc = nc.dram_tensor("onc", [P, 1], f32, kind="ExternalInput")
    nonr = nc.dram_tensor("nonr", [1, P], f32, kind="ExternalInput")
    out = nc.dram_tensor("out", [CT, P, TQ], f32, kind="ExternalOutput")

    with tile.TileContext(nc) as tc, ExitStack() as ctx:
        const = ctx.enter_context(tc.tile_pool(name="const", bufs=1))

        wq_sb = const.tile([P, CT, C], f32r, tag="wq")
        wp_sb = const.tile([P, CT, C], f32r, tag="wp")
        k_sb = const.tile([P, CT, T], bf16, tag="k")
        v_sb = const.tile([P, NST, C], bf16, tag="v")
        mask_sb = const.tile([P, 4, SUB], bf16, tag="mask")
        bvb_sb = const.tile([P, C], f32, tag="bvb")
        bq_sb = const.tile([P, CT], f32, tag="bq")
        bk_sb = const.tile([P, CT], f32, tag="bk")
        ones_c = const.tile([P, 1], f32r, tag="onec")
        onr_sb = const.tile([1, P], f32r, tag="onr")

        # PSUM allocates whole 2KB banks per buffer: mm 4 + ht 4 = 8 banks.
        # One deep rotating pool for every non-ht matmul output avoids
        # buffer-reuse waits between the finish tail and the next chunk.
        pp = ctx.enter_context(tc.tile_pool(name="pp", bufs=4, space="PSUM"))
        ph = ctx.enter_context(tc.tile_pool(name="ph", bufs=1, space="PSUM"))

        qp = ctx.enter_context(tc.tile_pool(name="qp", bufs=2))
        ep = ctx.enter_context(tc.tile_pool(name="ep", bufs=10))
        hp = ctx.enter_context(tc.tile_pool(name="hp", bufs=1))
        op = ctx.enter_context(tc.tile_pool(name="op", bufs=2))
        ogp = ctx.enter_context(tc.tile_pool(name="ogp", bufs=3))

        # phase-2 constants, loaded while phase-1 compute runs (dribbled a
        # few per s-chunk so they never sit ahead of the critical x loads)
        late_dmas = []
        late_dmas.append(lambda: nc.sync.dma_start(
            wq_sb[:], wqt[:].bitcast(f32r)))
        late_dmas.append(lambda: nc.sync.dma_start(bq_sb[:], bqd[:]))
        for m in range(4):
            late_dmas.append(lambda m=m: nc.sync.dma_start(
                mask_sb[:, m, :], mkd[m]))
        late_dmas.append(lambda: nc.sync.dma_start(
            ones_c[:], onc[:].bitcast(f32r)))
        late_dmas.append(lambda: nc.sync.dma_start(
            onr_sb[:], nonr[:].bitcast(f32r)))
        late_dmas.append(lambda: nc.sync.dma_start(
            wp_sb[:], wpt[:].bitcast(f32r)))

        chunk_tiles = {}

        def load_q(c):
            qx = qp.tile([P, CT, SUB], f32r, tag="qx", name="qx")
            nc.sync.dma_start(qx[:], xq[:, :, ts(c, SUB)].bitcast(f32r))
            q_sb = qp.tile([P, CT, SUB], bf16, tag="q", name="q_sb")
            for o in range(CT):
                pq = pp.tile([P, SUB], f32, tag="mm", name="pq")
                for cj in range(CT):
                    nc.tensor.matmul(pq[:], wq_sb[:, cj, ts(o, P)],
                                     qx[:, cj, :],
                                     start=(cj == 0), stop=(cj == CT - 1))
                nc.scalar.activation(q_sb[:, o, :], pq[:], AF.Identity,
                                     bias=bq_sb[:, o:o + 1])
            chunk_tiles[c] = (q_sb, qx)

        # ---- phase 1: K and V resident in SBUF --------------------------
        with tc.tile_pool(name="xp", bufs=2) as xp, \
             tc.tile_pool(name="wkv", bufs=1) as wkv:
            wk_sb = wkv.tile([P, CT, C], f32r, tag="wk")
            wv_sb = wkv.tile([P, CT, C], f32r, tag="wv")
            nc.sync.dma_start(wk_sb[:], wkt[:].bitcast(f32r))
            for sc in range(NCH):
                xt = xp.tile([P, CT, 512], f32r, tag="xt")
                nc.sync.dma_start(xt[:],
                                  xb[:, :, ts(sc, 512)].bitcast(f32r))
                if sc == 0:
                    # needed from the first V tile onward; after the first
                    # x chunk so the very first K matmul starts asap
                    nc.sync.dma_start(wv_sb[:], wvt[:].bitcast(f32r))
                    nc.sync.dma_start(bk_sb[:], bkd[:])
                    nc.sync.dma_start(bvb_sb[:], bvd[:])
                for o in range(CT):
                    pk = pp.tile([P, 512], f32, tag="mm", name="pk")
                    for cj in range(CT):
                        nc.tensor.matmul(pk[:], wk_sb[:, cj, ts(o, P)],
                                         xt[:, cj, :],
                                         start=(cj == 0), stop=(cj == CT - 1))
                    nc.scalar.activation(k_sb[:, o, ts(sc, 512)], pk[:],
                                         AF.Identity, bias=bk_sb[:, o:o + 1])
                for ss in range(4):
                    pv = pp.tile([P, 512], f32, tag="mm", name="pv")
                    for cj in range(CT):
                        nc.tensor.matmul(pv[:], xt[:, cj, ts(ss, P)],
                                         wv_sb[:, cj, :],
                                         start=(cj == 0), stop=(cj == CT - 1))
                    nc.vector.tensor_add(v_sb[:, sc * 4 + ss, :], pv[:],
                                         bvb_sb[:])
                if sc >= 1:
                    for _ in range(4):
                        if late_dmas:
                            late_dmas.pop(0)()
                if sc == NCH - 2:
                    while late_dmas:
                        late_dmas.pop(0)()
                    # chunk-0 query prefetch + projection: the DMA lands and
                    # the matmuls queue behind the remaining phase-1 work,
                    # so phase 2 starts with zero PE idle
                    load_q(0)

        # ---- phase 2: attention per query chunk, pipelined across chunks
        def s_loop(c):
            q_sb, _ = chunk_tiles[c]
            ntr = 4 * c + 4
            ht = [ph.tile([P, SUB], f32, tag=f"ht{cs}", name=f"ht{cs}")
                  for cs in range(CT)]
            acc = op.tile([P, SUB], f32r, tag="acc", name="acc")
            st_tiles = {}

            def qk(k):
                stp = pp.tile([P, SUB], f32, tag="mm", name="stp")
                for cj in range(CT):
                    nc.tensor.matmul(stp[:], k_sb[:, cj, ts(k, P)],
                                     q_sb[:, cj, :],
                                     start=(cj == 0), stop=(cj == CT - 1))
                st_tiles[k] = stp

            qk(0)
            for k in range(ntr):
                if k + 1 < ntr:
                    qk(k + 1)
                stp = st_tiles.pop(k)
                et = ep.tile([P, SUB], bf16, tag="et", name="et")
                nc.scalar.activation(et[:], stp[:], AF.Exp, scale=SCALE)
                if k >= 4 * c:
                    nc.vector.tensor_mul(et[:], et[:],
                                         mask_sb[:, k - 4 * c, :])
                if k == 0:
                    nc.vector.tensor_copy(acc[:], et[:])
                else:
                    nc.vector.tensor_add(acc[:], acc[:], et[:])
                for cs in range(CT):
                    nc.tensor.matmul(ht[cs][:], v_sb[:, k, ts(cs, P)],
                                     et[:], start=(k == 0),
                                     stop=(k == ntr - 1))
            # drain ht -> SBUF right away (queued on ACT before the next
            # chunk's q-bias ops) so the out projection never waits
            hs = hp.tile([P, CT, SUB], f32r, tag="hs", name="hs")
            for cs in range(CT):
                nc.scalar.activation(hs[:, cs, :], ht[cs][:], AF.Identity)
            return hs, acc

        def finish_head(c, acc):
            # softmax denominator: partition-sum of acc via a ones matmul,
            # then reciprocal on DVE (emitted before load_q so it overlaps
            # the next chunk's Q projection on the PE)
            sm = pp.tile([1, SUB], f32, tag="mm", name="sm")
            nc.tensor.matmul(sm[:], ones_c[:], acc[:],
                             start=True, stop=True)
            r_sb = op.tile([1, SUB], f32r, tag="r", name="r_sb")
            with nc.allow_low_precision(reason="float32r is fp32-width"):
                nc.vector.reciprocal(r_sb[:], sm[:])
            return r_sb

        def finish(c, hs, r_sb):
            _, qx = chunk_tiles.pop(c)
            prb = pp.tile([P, SUB], f32, tag="mm", name="prb")
            nc.tensor.matmul(prb[:], onr_sb[:], r_sb[:], start=True,
                             stop=True)
            r_b = op.tile([P, SUB], f32, tag="rb", name="r_b")
            nc.scalar.activation(r_b[:], prb[:], AF.Identity)

            for o in range(CT):
                pu = pp.tile([P, SUB], f32, tag="mm", name="pu")
                for cj in range(CT):
                    nc.tensor.matmul(pu[:], wp_sb[:, cj, ts(o, P)],
                                     hs[:, cj, :],
                                     start=(cj == 0), stop=(cj == CT - 1))
                og = ogp.tile([P, SUB], f32, tag="og", name="og")
                nc.vector.tensor_mul(og[:], pu[:], r_b[:])
                nc.vector.tensor_add(og[:], og[:], qx[:, o, :].bitcast(f32))
                nc.sync.dma_start(out[o][:, ts(c, SUB)], og[:])

        for c in range(NCH):
            hs, acc = s_loop(c)
            r_sb = finish_head(c, acc)
            if c + 1 < NCH:
                # next chunk's Q projection keeps the PE busy while the
                # denominator drains through DVE
                load_q(c + 1)
            finish(c, hs, r_sb)

    nc.finalize()
    return nc


def _masks(h):
    m = np.zeros((4, P, SUB), np.float32)
    f = np.arange(SUB)[None, :]
    p = np.arange(P)[:, None]
    for k in range(4):
        d = 128 * k - 256 * h
        m[k] = (f >= p + d).astype(np.float32)
    return m.astype(ml_dtypes.bfloat16)


def _in_maps(inputs):
    x = np.asarray(inputs["x"], np.float32)
    Wq = np.asarray(inputs["Wq"], np.float64)
    bq = np.asarray(inputs["bq"], np.float64)
    Wk = np.asarray(inputs["Wk"], np.float32)
    bk = np.asarray(inputs["bk"], np.float32)
    Wv = np.asarray(inputs["Wv"], np.float32)
    bv = np.asarray(inputs["bv"], np.float32)
    Wp = np.asarray(inputs["Wp"], np.float32)
    bp = np.asarray(inputs["bp"], np.float64)

    bqp = (bq - Wq @ bp).astype(np.float32)   # bq' = bq - Wq bp
    Wq32 = Wq.astype(np.float32)
    bp32 = bp.astype(np.float32)

    common = {
        "wqt": np.ascontiguousarray(Wq32.T.reshape(CT, P, C).transpose(1, 0, 2)),
        "wkt": np.ascontiguousarray(Wk.T.reshape(CT, P, C).transpose(1, 0, 2)),
        "wvt": np.ascontiguousarray(Wv.T.reshape(CT, P, C).transpose(1, 0, 2)),
        "wpt": np.ascontiguousarray(Wp.T.reshape(CT, P, C).transpose(1, 0, 2)),
        "bqd": np.ascontiguousarray(bqp.reshape(CT, P).T),
        "bkd": np.ascontiguousarray(bk.reshape(CT, P).T),
        "bvd": np.ascontiguousarray(np.broadcast_to(bv[None, :], (P, C))),
        "onc": np.ones((P, 1), np.float32),
        "nonr": np.ones((1, P), np.float32),
    }
    maps = []
    for core in range(NCORE):
        b, h = divmod(core, 2)
        cols = (np.arange(NCH)[:, None] * 512 + h * SUB
                + np.arange(SUB)[None, :]).ravel()
        xg = x[b][:, cols] + bp32[:, None]      # [C, TQ] queries + bp
        m = dict(common)
        m["xb"] = np.ascontiguousarray(
            x[b].reshape(CT, P, T).transpose(1, 0, 2))
        m["xq"] = np.ascontiguousarray(
            xg.reshape(CT, P, TQ).transpose(1, 0, 2))
        m["mkd"] = _masks(h)
        maps.append((m, b, cols))
    return maps


_prog_cache = {}


def _get_program():
    if "nc" not in _prog_cache:
        _prog_cache["nc"] = _build_program()
    return _prog_cache["nc"]


def kernel(**inputs):
    return _run(inputs, trace=False)[0]


def _run(inputs, trace=False):
    nc = _get_program()
    maps = _in_maps(inputs)
    res = run_bass_kernel_spmd(nc, [m for m, _, _ in maps],
                               core_ids=list(range(NCORE)), trace=trace)
    x = np.asarray(inputs["x"], np.float32)
    full = np.empty((B, C, T), np.float32)
    for core, (_, b, cols) in enumerate(maps):
        full[b][:, cols] = res.results[core]["out"].reshape(C, TQ)
    return full, res
